# revision 20
# baseline (speedup 1.0000x reference)
"""Trainium2 Bass kernel for nn_BackboneModel (backbone frame rebuild).

The reference scatters rows into a padded [B, L, 14, 3] block, builds
Gram-Schmidt rigid frames from (N, CA, C), places ideal N/CA/C/O atoms,
and gathers the valid rows back.  Scatter followed by gather at the same
(batch_id, pos) indices is an identity permutation over the valid rows,
so the whole model is a pure per-row function of X[i]:

    e1 = normalize(C - CA)                      (normalize: v * rsqrt(|v|^2 + eps^2))
    e2 = normalize((N - CA) - ((N - CA).e1) e1)
    out[0] = -0.525*e1 + 1.363*e2 + CA          (N)
    out[1] = CA                                 (CA, passthrough)
    out[2] =  1.526*e1            + CA          (C)
    out[3] =  2.153*e1 - 1.062*e2 + CA          (O)
    out[4:14] = X[4:14]                         (passthrough)

Only atoms 0..2 (N, CA, C) feed any arithmetic, and only atoms 0, 2, 3
get new values.  The device reads a packed [rows, 9] f32 block (N, CA,
C) and writes a packed atom-major [3, rows, 3] bf16 block (outN, outC,
outO); CA and atoms 4..13 are stitched in on the host during unshard
(pure data movement).  Correctness gate is rel-L2 < 2e-2; bf16 rounding
of final coordinates contributes ~1.4e-3.

Numerics: the rejection w = v - (v.d1/|d1|^2) d1 amplifies projection
error by |v|/|w|, so that chain stays f32 with the 18-bit DVE
reciprocal-approx.  rs1/rs2 only scale outputs (ACT table Sqrt ok), and
the placement chain runs bf16 (measured DVE modes: bf16 tensor_scalar
4x, bf16 tensor_tensor 2x, everything else 1x).  Per-row scalars are
materialized to dense bf16 on ACT because broadcast operands force 1x.
The reference's +eps^2 regularizer is dropped (inputs are randn;
relative effect ~1e-8).

Engine schedule (measured costs drove this):  Pool does the two input
subtractions; ACT does squares, sqrts, bf16 shadows/materializations
and store issue; DVE does products, reduces, reciprocal and the bf16
tail.  Emission is modulo-scheduled: each round interleaves tile r's
head chain with tile r-1's tail ops so the in-order DVE stream stays
busy during cross-engine hops.

Sharding: data-parallel, 8 equal contiguous row chunks of 98304 rows.
"""

import numpy as np

N_CORES = 8
N_TOTAL = 786432
N_CORE = N_TOTAL // N_CORES      # 98304 rows per core
P = 128                          # SBUF partitions
ROWS_PER_PART = N_CORE // P      # 768 rows per partition per core
TILE_SIZES = [64, 384, 320]
CIN = 9                          # N, CA, C xyz (f32)

_NC = None


def _build_nc():
    import concourse.bacc as bacc
    import concourse.tile as tile
    from concourse import mybir

    f32 = mybir.dt.float32
    bf16 = mybir.dt.bfloat16
    AX = mybir.AxisListType.X
    SQRT = mybir.ActivationFunctionType.Sqrt
    SQUARE = mybir.ActivationFunctionType.Square
    COPY = mybir.ActivationFunctionType.Copy
    ARSQRT = mybir.ActivationFunctionType.Abs_reciprocal_sqrt

    nc = bacc.Bacc()
    X = nc.declare_dram_parameter("X", [N_CORE, CIN], f32, isOutput=False)
    Y = nc.declare_dram_parameter("Y", [3, N_CORE, 3], bf16, isOutput=True)

    def bcast(s, r):  # [P, r] per-row scalar -> [P, r, 3]
        return s[:, :, None].broadcast_to([P, r, 3])

    n = len(TILE_SIZES)
    offs = []
    o = 0
    for R in TILE_SIZES:
        offs.append(o)
        o += P * R
    assert o == N_CORE

    with tile.TileContext(nc) as tc:
        with tc.tile_pool(name="io", bufs=3) as io, \
             tc.tile_pool(name="dv", bufs=3) as dv, \
             tc.tile_pool(name="v3", bufs=2) as v3, \
             tc.tile_pool(name="sc", bufs=2) as sc:
            st = [dict(R=R, off=offs[i]) for i, R in enumerate(TILE_SIZES)]

            def emit_load(i):
                s = st[i]
                R = s["R"]
                s["T"] = io.tile([P, R, CIN], f32, tag="T", name="T")
                nc.sync.dma_start(
                    out=s["T"],
                    in_=X[s["off"]:s["off"] + P * R, :].rearrange(
                        "(p r) c -> p r c", p=P))

            def emit_subs(i):
                s = st[i]
                R = s["R"]
                T = s["T"]
                s["D1"] = dv.tile([P, R, 3], f32, tag="d1", name="d1")
                s["V"] = dv.tile([P, R, 3], f32, tag="v", name="v")
                # tile 0 subs on DVE: shortens the cold-start chain (DVE
                # is idle then anyway); later tiles use the Pool engine.
                eng = nc.vector if i == 0 else nc.gpsimd
                eng.tensor_sub(s["D1"], T[:, :, 6:9], T[:, :, 3:6])
                eng.tensor_sub(s["V"], T[:, :, 0:3], T[:, :, 3:6])

            def alloc_head(i):
                s = st[i]
                R = s["R"]
                s["SQ1"] = v3.tile([P, R, 3], f32, tag="sq1", name="sq1")
                s["P2"] = v3.tile([P, R, 3], bf16, tag="p2", name="p2")
                s["Vb"] = v3.tile([P, R, 3], bf16, tag="vb", name="vb")
                s["T1"] = v3.tile([P, R, 3], f32, tag="t1", name="t1")
                s["W"] = v3.tile([P, R, 3], f32, tag="w", name="w")
                s["SQ2"] = v3.tile([P, R, 3], f32, tag="sq2", name="sq2")
                s["CAb"] = v3.tile([P, R, 3], bf16, tag="cab", name="cab")
                s["D1b"] = v3.tile([P, R, 3], bf16, tag="d1b", name="d1b")
                s["Wb"] = v3.tile([P, R, 3], bf16, tag="wb", name="wb")
                s["RS1d"] = v3.tile([P, R, 3], bf16, tag="rs1d", name="rs1d")
                s["RS2d"] = v3.tile([P, R, 3], bf16, tag="rs2d", name="rs2d")
                s["S1"] = sc.tile([P, R], f32, tag="s1", name="s1")
                s["DOT"] = sc.tile([P, R], f32, tag="dot", name="dot")
                s["IS1"] = sc.tile([P, R], f32, tag="is1", name="is1")
                s["G"] = sc.tile([P, R], f32, tag="g", name="g")
                s["S2"] = sc.tile([P, R], f32, tag="s2", name="s2")

            def alloc_tail(i):
                s = st[i]
                R = s["R"]
                s["E1"] = v3.tile([P, R, 3], bf16, tag="e1", name="e1")
                s["E2"] = v3.tile([P, R, 3], bf16, tag="e2", name="e2")
                s["SA"] = v3.tile([P, R, 3], bf16, tag="sa", name="sa")
                s["SB"] = v3.tile([P, R, 3], bf16, tag="sb", name="sb")
                s["TN"] = v3.tile([P, R, 3], bf16, tag="tn", name="tn")
                s["TO"] = v3.tile([P, R, 3], bf16, tag="to", name="to")
                s["OUT"] = io.tile([P, 3, R, 3], bf16, tag="OUT", name="OUT")

            def tail_a(i):
                # E1/E2 products + C-atom chain (fills ACT rs1 hop)
                if i < 0:
                    return
                s = st[i]
                nc.vector.tensor_mul(s["E1"], s["D1b"], s["RS1d"])
                nc.vector.tensor_mul(s["E2"], s["Wb"], s["RS2d"])
                nc.vector.tensor_scalar_mul(out=s["SA"], in0=s["E1"],
                                            scalar1=1.526)
                nc.vector.tensor_add(s["OUT"][:, 1], s["SA"], s["CAb"])

            def tail_b(i):
                # remaining placement chain (fills ACT sq2/wb hop)
                if i < 0:
                    return
                s = st[i]
                nc.vector.tensor_scalar_mul(out=s["SB"], in0=s["E2"],
                                            scalar1=1.363)
                nc.vector.tensor_add(s["TN"], s["SB"], s["CAb"])
                nc.vector.tensor_scalar_mul(out=s["SA"], in0=s["E1"],
                                            scalar1=-0.525)
                nc.vector.tensor_add(s["OUT"][:, 0], s["SA"], s["TN"])
                nc.vector.tensor_scalar_mul(out=s["SB"], in0=s["E2"],
                                            scalar1=-1.062)
                nc.vector.tensor_add(s["TO"], s["SB"], s["CAb"])
                nc.vector.tensor_scalar_mul(out=s["SA"], in0=s["E1"],
                                            scalar1=2.153)
                nc.vector.tensor_add(s["OUT"][:, 2], s["SA"], s["TO"])

            def emit_store(i, atom=None):
                if i < 0:
                    return
                s = st[i]
                if atom is None:
                    nc.scalar.dma_start(
                        out=Y[:, s["off"]:s["off"] + P * s["R"], :].rearrange(
                            "a (p r) c -> p a r c", p=P),
                        in_=s["OUT"])
                else:
                    nc.scalar.dma_start(
                        out=Y[atom, s["off"]:s["off"] + P * s["R"], :]
                        .rearrange("(p r) c -> p r c", p=P),
                        in_=s["OUT"][:, atom])

            # ---- prologue ----
            emit_load(0)
            emit_subs(0)

            for r in range(n):
                s = st[r]
                R = s["R"]
                alloc_head(r)
                if r > 0:
                    alloc_tail(r - 1)
                if r + 1 < n:
                    emit_load(r + 1)
                    emit_subs(r + 1)
                # s1 & dot (dot products in bf16 at 2x; squares f32 on ACT)
                nc.scalar.activation(out=s["SQ1"], in_=s["D1"], func=SQUARE)
                nc.scalar.activation(out=s["CAb"], in_=s["T"][:, :, 3:6],
                                     func=COPY)
                nc.scalar.activation(out=s["D1b"], in_=s["D1"], func=COPY)
                nc.scalar.activation(out=s["Vb"], in_=s["V"], func=COPY)
                nc.vector.tensor_mul(s["P2"], s["D1b"], s["Vb"])
                nc.vector.reduce_sum(out=s["S1"], in_=s["SQ1"], axis=AX)
                nc.vector.reduce_sum(out=s["DOT"], in_=s["P2"], axis=AX)
                nc.vector.reciprocal_approx_fast(out=s["IS1"], in_=s["S1"])
                nc.scalar.activation(out=s["RS1d"], in_=bcast(s["IS1"], R),
                                     func=SQRT)
                # fill DVE while ACT does the rs1 chain
                tail_a(r - 1)
                # rejection
                nc.vector.tensor_mul(s["G"], s["DOT"], s["IS1"])
                nc.vector.tensor_mul(s["T1"], s["D1"], bcast(s["G"], R))
                nc.vector.tensor_sub(s["W"], s["V"], s["T1"])
                nc.scalar.activation(out=s["SQ2"], in_=s["W"], func=SQUARE)
                nc.scalar.activation(out=s["Wb"], in_=s["W"], func=COPY)
                # fill DVE while ACT does sq2/wb
                tail_b(r - 1)
                emit_store(r - 1)
                nc.vector.reduce_sum(out=s["S2"], in_=s["SQ2"], axis=AX)
                nc.scalar.activation(out=s["RS2d"], in_=bcast(s["S2"], R),
                                     func=ARSQRT)

            # ---- epilogue: last tile's tail (stores split per atom) ----
            alloc_tail(n - 1)
            tail_a(n - 1)
            emit_store(n - 1, atom=1)
            tail_b(n - 1)
            emit_store(n - 1, atom=0)
            emit_store(n - 1, atom=2)
    nc.finalize()
    return nc


def _get_nc():
    global _NC
    if _NC is None:
        _NC = _build_nc()
    return _NC


def make_in_maps(X):
    """Pack (N, CA, C) as contiguous [N_CORE, 9] f32 shards per core."""
    X = np.asarray(X, dtype=np.float32)
    A = np.ascontiguousarray(X[:, 0:3, :]).reshape(N_TOTAL, CIN)
    shards = A.reshape(N_CORES, N_CORE, CIN)
    return [{"X": shards[c]} for c in range(N_CORES)]


def assemble(X, results):
    """Stitch device outputs (atom-major bf16 [3, rows, 3]) into the
    full [N, 14, 3] f32 array."""
    out = np.array(X, dtype=np.float32, copy=True)
    Ys = [np.asarray(results[c]["Y"]).astype(np.float32)
          for c in range(N_CORES)]
    Yall = np.concatenate(Ys, axis=1)  # [3, N_TOTAL, 3]
    out[:, 0, :] = Yall[0]
    out[:, 2, :] = Yall[1]
    out[:, 3, :] = Yall[2]
    return out


def kernel(X, batch_ids=None, max_len=None, **_unused):
    from concourse.bass_utils import run_bass_kernel_spmd

    X = np.asarray(X, dtype=np.float32)
    assert X.shape == (N_TOTAL, 14, 3), X.shape
    nc = _get_nc()
    res = run_bass_kernel_spmd(nc, make_in_maps(X), list(range(N_CORES))).results
    return assemble(X, res)


# revision 22
# speedup vs baseline: 1.0080x; 1.0080x over previous
"""Trainium2 Bass kernel for nn_BackboneModel (backbone frame rebuild).

The reference scatters rows into a padded [B, L, 14, 3] block, builds
Gram-Schmidt rigid frames from (N, CA, C), places ideal N/CA/C/O atoms,
and gathers the valid rows back.  Scatter followed by gather at the same
(batch_id, pos) indices is an identity permutation over the valid rows,
so the whole model is a pure per-row function of X[i]:

    e1 = normalize(C - CA)                      (normalize: v * rsqrt(|v|^2 + eps^2))
    e2 = normalize((N - CA) - ((N - CA).e1) e1)
    out[0] = -0.525*e1 + 1.363*e2 + CA          (N)
    out[1] = CA                                 (CA, passthrough)
    out[2] =  1.526*e1            + CA          (C)
    out[3] =  2.153*e1 - 1.062*e2 + CA          (O)
    out[4:14] = X[4:14]                         (passthrough)

Only atoms 0..2 (N, CA, C) feed any arithmetic, and only atoms 0, 2, 3
get new values.  The device reads a packed [rows, 9] f32 block (N, CA,
C) and writes a packed atom-major [3, rows, 3] bf16 block (outN, outC,
outO); CA and atoms 4..13 are stitched in on the host during unshard
(pure data movement).  Correctness gate is rel-L2 < 2e-2; bf16 rounding
of final coordinates contributes ~1.4e-3.

Numerics: the rejection w = v - (v.d1/|d1|^2) d1 amplifies projection
error by |v|/|w|, so that chain stays f32 with the 18-bit DVE
reciprocal-approx.  rs1/rs2 only scale outputs (ACT table Sqrt ok), and
the placement chain runs bf16 (measured DVE modes: bf16 tensor_scalar
4x, bf16 tensor_tensor 2x, everything else 1x).  Per-row scalars are
materialized to dense bf16 on ACT because broadcast operands force 1x.
The reference's +eps^2 regularizer is dropped (inputs are randn;
relative effect ~1e-8).

Engine schedule (measured costs drove this):  Pool does the two input
subtractions; ACT does squares, sqrts, bf16 shadows/materializations
and store issue; DVE does products, reduces, reciprocal and the bf16
tail.  Emission is modulo-scheduled: each round interleaves tile r's
head chain with tile r-1's tail ops so the in-order DVE stream stays
busy during cross-engine hops.

Sharding: data-parallel, 8 equal contiguous row chunks of 98304 rows.
"""

import numpy as np

N_CORES = 8
N_TOTAL = 786432
N_CORE = N_TOTAL // N_CORES      # 98304 rows per core
P = 128                          # SBUF partitions
ROWS_PER_PART = N_CORE // P      # 768 rows per partition per core
TILE_SIZES = [64, 384, 320]
CIN = 9                          # N, CA, C xyz (f32)

_NC = None


def _build_nc():
    import concourse.bacc as bacc
    import concourse.tile as tile
    from concourse import mybir

    f32 = mybir.dt.float32
    bf16 = mybir.dt.bfloat16
    AX = mybir.AxisListType.X
    SQRT = mybir.ActivationFunctionType.Sqrt
    SQUARE = mybir.ActivationFunctionType.Square
    COPY = mybir.ActivationFunctionType.Copy

    nc = bacc.Bacc()
    X = nc.declare_dram_parameter("X", [N_CORE, CIN], f32, isOutput=False)
    Y = nc.declare_dram_parameter("Y", [3, N_CORE, 3], bf16, isOutput=True)

    def bcast(s, r):  # [P, r] per-row scalar -> [P, r, 3]
        return s[:, :, None].broadcast_to([P, r, 3])

    n = len(TILE_SIZES)
    offs = []
    o = 0
    for R in TILE_SIZES:
        offs.append(o)
        o += P * R
    assert o == N_CORE

    with tile.TileContext(nc) as tc:
        with tc.tile_pool(name="io", bufs=3) as io, \
             tc.tile_pool(name="dv", bufs=3) as dv, \
             tc.tile_pool(name="v3", bufs=2) as v3, \
             tc.tile_pool(name="sc", bufs=2) as sc:
            st = [dict(R=R, off=offs[i]) for i, R in enumerate(TILE_SIZES)]

            # Warm the ACT Square/Sqrt tables at emission start so both
            # ACT_TABLE_LOADs overlap the framework preamble / first DMA
            # instead of landing in tile 0's critical chain.
            wt = sc.tile([P, 1], f32, tag="wt", name="wt")
            wu = sc.tile([P, 1], f32, tag="wu", name="wu")
            nc.gpsimd.memset(wt, 1.0)
            nc.scalar.activation(out=wu, in_=wt, func=SQUARE)
            nc.scalar.activation(out=wt, in_=wu, func=SQRT)

            def emit_load(i):
                s = st[i]
                R = s["R"]
                s["T"] = io.tile([P, R, CIN], f32, tag="T", name="T")
                nc.sync.dma_start(
                    out=s["T"],
                    in_=X[s["off"]:s["off"] + P * R, :].rearrange(
                        "(p r) c -> p r c", p=P))

            def emit_subs(i):
                s = st[i]
                R = s["R"]
                T = s["T"]
                s["D1"] = dv.tile([P, R, 3], f32, tag="d1", name="d1")
                s["V"] = dv.tile([P, R, 3], f32, tag="v", name="v")
                # tile 0 subs on DVE: shortens the cold-start chain (DVE
                # is idle then anyway); later tiles use the Pool engine.
                eng = nc.vector if i == 0 else nc.gpsimd
                eng.tensor_sub(s["D1"], T[:, :, 6:9], T[:, :, 3:6])
                eng.tensor_sub(s["V"], T[:, :, 0:3], T[:, :, 3:6])

            def alloc_head(i):
                s = st[i]
                R = s["R"]
                s["SQ1"] = v3.tile([P, R, 3], f32, tag="sq1", name="sq1")
                s["P2"] = v3.tile([P, R, 3], bf16, tag="p2", name="p2")
                s["Vb"] = v3.tile([P, R, 3], bf16, tag="vb", name="vb")
                s["T1"] = v3.tile([P, R, 3], f32, tag="t1", name="t1")
                s["W"] = v3.tile([P, R, 3], f32, tag="w", name="w")
                s["SQ2"] = v3.tile([P, R, 3], f32, tag="sq2", name="sq2")
                s["CAb"] = v3.tile([P, R, 3], bf16, tag="cab", name="cab")
                s["D1b"] = v3.tile([P, R, 3], bf16, tag="d1b", name="d1b")
                s["Wb"] = v3.tile([P, R, 3], bf16, tag="wb", name="wb")
                s["RS1d"] = v3.tile([P, R, 3], bf16, tag="rs1d", name="rs1d")
                s["RS2d"] = v3.tile([P, R, 3], bf16, tag="rs2d", name="rs2d")
                s["S1"] = sc.tile([P, R], f32, tag="s1", name="s1")
                s["DOT"] = sc.tile([P, R], f32, tag="dot", name="dot")
                s["IS1"] = sc.tile([P, R], f32, tag="is1", name="is1")
                s["G"] = sc.tile([P, R], f32, tag="g", name="g")
                s["S2"] = sc.tile([P, R], f32, tag="s2", name="s2")
                s["IS2"] = sc.tile([P, R], f32, tag="is2", name="is2")

            def alloc_tail(i):
                s = st[i]
                R = s["R"]
                s["E1"] = v3.tile([P, R, 3], bf16, tag="e1", name="e1")
                s["E2"] = v3.tile([P, R, 3], bf16, tag="e2", name="e2")
                s["SA"] = v3.tile([P, R, 3], bf16, tag="sa", name="sa")
                s["SB"] = v3.tile([P, R, 3], bf16, tag="sb", name="sb")
                s["TN"] = v3.tile([P, R, 3], bf16, tag="tn", name="tn")
                s["TO"] = v3.tile([P, R, 3], bf16, tag="to", name="to")
                s["OUT"] = io.tile([P, 3, R, 3], bf16, tag="OUT", name="OUT")

            def tail_a(i):
                # E1/E2 products + C-atom chain (fills ACT rs1 hop)
                if i < 0:
                    return
                s = st[i]
                nc.vector.tensor_mul(s["E1"], s["D1b"], s["RS1d"])
                nc.vector.tensor_mul(s["E2"], s["Wb"], s["RS2d"])
                nc.vector.tensor_scalar_mul(out=s["SA"], in0=s["E1"],
                                            scalar1=1.526)
                nc.vector.tensor_add(s["OUT"][:, 1], s["SA"], s["CAb"])

            def tail_b(i):
                # remaining placement chain (fills ACT sq2/wb hop)
                if i < 0:
                    return
                s = st[i]
                nc.vector.tensor_scalar_mul(out=s["SB"], in0=s["E2"],
                                            scalar1=1.363)
                nc.vector.tensor_add(s["TN"], s["SB"], s["CAb"])
                nc.vector.tensor_scalar_mul(out=s["SA"], in0=s["E1"],
                                            scalar1=-0.525)
                nc.vector.tensor_add(s["OUT"][:, 0], s["SA"], s["TN"])
                nc.vector.tensor_scalar_mul(out=s["SB"], in0=s["E2"],
                                            scalar1=-1.062)
                nc.vector.tensor_add(s["TO"], s["SB"], s["CAb"])
                nc.vector.tensor_scalar_mul(out=s["SA"], in0=s["E1"],
                                            scalar1=2.153)
                nc.vector.tensor_add(s["OUT"][:, 2], s["SA"], s["TO"])

            def emit_store(i, atom=None):
                if i < 0:
                    return
                s = st[i]
                if atom is None:
                    nc.scalar.dma_start(
                        out=Y[:, s["off"]:s["off"] + P * s["R"], :].rearrange(
                            "a (p r) c -> p a r c", p=P),
                        in_=s["OUT"])
                else:
                    nc.scalar.dma_start(
                        out=Y[atom, s["off"]:s["off"] + P * s["R"], :]
                        .rearrange("(p r) c -> p r c", p=P),
                        in_=s["OUT"][:, atom])

            # ---- prologue ----
            emit_load(0)
            emit_subs(0)

            for r in range(n):
                s = st[r]
                R = s["R"]
                alloc_head(r)
                if r > 0:
                    alloc_tail(r - 1)
                if r + 1 < n:
                    emit_load(r + 1)
                    emit_subs(r + 1)
                # s1 & dot (dot products in bf16 at 2x; squares f32 on ACT)
                nc.scalar.activation(out=s["SQ1"], in_=s["D1"], func=SQUARE)
                nc.scalar.activation(out=s["CAb"], in_=s["T"][:, :, 3:6],
                                     func=COPY)
                nc.scalar.activation(out=s["D1b"], in_=s["D1"], func=COPY)
                nc.scalar.activation(out=s["Vb"], in_=s["V"], func=COPY)
                nc.vector.tensor_mul(s["P2"], s["D1b"], s["Vb"])
                nc.vector.reduce_sum(out=s["S1"], in_=s["SQ1"], axis=AX)
                nc.vector.reduce_sum(out=s["DOT"], in_=s["P2"], axis=AX)
                nc.vector.reciprocal_approx_fast(out=s["IS1"], in_=s["S1"])
                nc.scalar.activation(out=s["RS1d"], in_=bcast(s["IS1"], R),
                                     func=SQRT)
                # fill DVE while ACT does the rs1 chain
                tail_a(r - 1)
                # rejection
                nc.vector.tensor_mul(s["G"], s["DOT"], s["IS1"])
                nc.vector.tensor_mul(s["T1"], s["D1"], bcast(s["G"], R))
                nc.vector.tensor_sub(s["W"], s["V"], s["T1"])
                nc.scalar.activation(out=s["SQ2"], in_=s["W"], func=SQUARE)
                nc.scalar.activation(out=s["Wb"], in_=s["W"], func=COPY)
                # fill DVE while ACT does sq2/wb
                tail_b(r - 1)
                emit_store(r - 1)
                nc.vector.reduce_sum(out=s["S2"], in_=s["SQ2"], axis=AX)
                nc.vector.reciprocal_approx_fast(out=s["IS2"], in_=s["S2"])
                nc.scalar.activation(out=s["RS2d"], in_=bcast(s["IS2"], R),
                                     func=SQRT)

            # ---- epilogue: last tile's tail (stores split per atom) ----
            alloc_tail(n - 1)
            tail_a(n - 1)
            emit_store(n - 1, atom=1)
            tail_b(n - 1)
            emit_store(n - 1, atom=0)
            emit_store(n - 1, atom=2)
    nc.finalize()
    return nc


def _get_nc():
    global _NC
    if _NC is None:
        _NC = _build_nc()
    return _NC


def make_in_maps(X):
    """Pack (N, CA, C) as contiguous [N_CORE, 9] f32 shards per core."""
    X = np.asarray(X, dtype=np.float32)
    A = np.ascontiguousarray(X[:, 0:3, :]).reshape(N_TOTAL, CIN)
    shards = A.reshape(N_CORES, N_CORE, CIN)
    return [{"X": shards[c]} for c in range(N_CORES)]


def assemble(X, results):
    """Stitch device outputs (atom-major bf16 [3, rows, 3]) into the
    full [N, 14, 3] f32 array."""
    out = np.array(X, dtype=np.float32, copy=True)
    Ys = [np.asarray(results[c]["Y"]).astype(np.float32)
          for c in range(N_CORES)]
    Yall = np.concatenate(Ys, axis=1)  # [3, N_TOTAL, 3]
    out[:, 0, :] = Yall[0]
    out[:, 2, :] = Yall[1]
    out[:, 3, :] = Yall[2]
    return out


def kernel(X, batch_ids=None, max_len=None, **_unused):
    from concourse.bass_utils import run_bass_kernel_spmd

    X = np.asarray(X, dtype=np.float32)
    assert X.shape == (N_TOTAL, 14, 3), X.shape
    nc = _get_nc()
    res = run_bass_kernel_spmd(nc, make_in_maps(X), list(range(N_CORES))).results
    return assemble(X, res)


# revision 27
# speedup vs baseline: 1.0084x; 1.0004x over previous
"""Trainium2 Bass kernel for nn_BackboneModel (backbone frame rebuild).

The reference scatters rows into a padded [B, L, 14, 3] block, builds
Gram-Schmidt rigid frames from (N, CA, C), places ideal N/CA/C/O atoms,
and gathers the valid rows back.  Scatter followed by gather at the same
(batch_id, pos) indices is an identity permutation over the valid rows,
so the whole model is a pure per-row function of X[i]:

    e1 = normalize(C - CA)                      (normalize: v * rsqrt(|v|^2 + eps^2))
    e2 = normalize((N - CA) - ((N - CA).e1) e1)
    out[0] = -0.525*e1 + 1.363*e2 + CA          (N)
    out[1] = CA                                 (CA, passthrough)
    out[2] =  1.526*e1            + CA          (C)
    out[3] =  2.153*e1 - 1.062*e2 + CA          (O)
    out[4:14] = X[4:14]                         (passthrough)

Only atoms 0..2 (N, CA, C) feed any arithmetic, and only atoms 0, 2, 3
get new values.  The device reads a packed [rows, 9] f32 block (N, CA,
C) and writes a packed atom-major [3, rows, 3] bf16 block (outN, outC,
outO); CA and atoms 4..13 are stitched in on the host during unshard
(pure data movement).  Correctness gate is rel-L2 < 2e-2; bf16 rounding
of final coordinates contributes ~1.4e-3.

Numerics: the rejection w = v - (v.d1/|d1|^2) d1 amplifies projection
error by |v|/|w|, so that chain stays f32 with the 18-bit DVE
reciprocal-approx.  rs1/rs2 only scale outputs (ACT table Sqrt ok), and
the placement chain runs bf16 (measured DVE modes: bf16 tensor_scalar
4x, bf16 tensor_tensor 2x, everything else 1x).  Per-row scalars are
materialized to dense bf16 on ACT because broadcast operands force 1x.
The reference's +eps^2 regularizer is dropped (inputs are randn;
relative effect ~1e-8).

Engine schedule (measured costs drove this):  Pool does the two input
subtractions; ACT does squares, sqrts, bf16 shadows/materializations
and store issue; DVE does products, reduces, reciprocal and the bf16
tail.  Emission is modulo-scheduled: each round interleaves tile r's
head chain with tile r-1's tail ops so the in-order DVE stream stays
busy during cross-engine hops.

Sharding: data-parallel, 8 equal contiguous row chunks of 98304 rows.
"""

import numpy as np

N_CORES = 8
N_TOTAL = 786432
N_CORE = N_TOTAL // N_CORES      # 98304 rows per core
P = 128                          # SBUF partitions
ROWS_PER_PART = N_CORE // P      # 768 rows per partition per core
TILE_SIZES = [64, 384, 320]
CIN = 9                          # N, CA, C xyz (f32)

_NC = None


def _build_nc():
    import concourse.bacc as bacc
    import concourse.tile as tile
    from concourse import mybir

    f32 = mybir.dt.float32
    bf16 = mybir.dt.bfloat16
    AX = mybir.AxisListType.X
    SQRT = mybir.ActivationFunctionType.Sqrt
    SQUARE = mybir.ActivationFunctionType.Square
    COPY = mybir.ActivationFunctionType.Copy

    nc = bacc.Bacc()
    X = nc.declare_dram_parameter("X", [N_CORE, CIN], f32, isOutput=False)
    Y = nc.declare_dram_parameter("Y", [3, N_CORE, 3], bf16, isOutput=True)

    def bcast(s, r):  # [P, r] per-row scalar -> [P, r, 3]
        return s[:, :, None].broadcast_to([P, r, 3])

    n = len(TILE_SIZES)
    offs = []
    o = 0
    for R in TILE_SIZES:
        offs.append(o)
        o += P * R
    assert o == N_CORE

    with tile.TileContext(nc) as tc:
        with tc.tile_pool(name="io", bufs=3) as io, \
             tc.tile_pool(name="dv", bufs=3) as dv, \
             tc.tile_pool(name="v3", bufs=2) as v3, \
             tc.tile_pool(name="sc", bufs=2) as sc:
            st = [dict(R=R, off=offs[i]) for i, R in enumerate(TILE_SIZES)]

            def emit_load(i):
                s = st[i]
                R = s["R"]
                s["T"] = io.tile([P, R, CIN], f32, tag="T", name="T")
                nc.sync.dma_start(
                    out=s["T"],
                    in_=X[s["off"]:s["off"] + P * R, :].rearrange(
                        "(p r) c -> p r c", p=P))

            def emit_subs(i):
                s = st[i]
                R = s["R"]
                T = s["T"]
                s["D1"] = dv.tile([P, R, 3], f32, tag="d1", name="d1")
                s["V"] = dv.tile([P, R, 3], f32, tag="v", name="v")
                # tile 0 subs on DVE: shortens the cold-start chain (DVE
                # is idle then anyway); later tiles use the Pool engine.
                eng = nc.vector if i == 0 else nc.gpsimd
                eng.tensor_sub(s["D1"], T[:, :, 6:9], T[:, :, 3:6])
                eng.tensor_sub(s["V"], T[:, :, 0:3], T[:, :, 3:6])

            def alloc_head(i):
                s = st[i]
                R = s["R"]
                s["QP"] = v3.tile([P, 2, R, 3], bf16, tag="qp", name="qp")
                s["Vb"] = v3.tile([P, R, 3], bf16, tag="vb", name="vb")
                s["T1"] = v3.tile([P, R, 3], f32, tag="t1", name="t1")
                s["W"] = v3.tile([P, R, 3], f32, tag="w", name="w")
                s["SQ2"] = v3.tile([P, R, 3], f32, tag="sq2", name="sq2")
                s["CAb"] = v3.tile([P, R, 3], bf16, tag="cab", name="cab")
                s["D1b"] = v3.tile([P, R, 3], bf16, tag="d1b", name="d1b")
                s["Wb"] = v3.tile([P, R, 3], bf16, tag="wb", name="wb")
                s["RS1d"] = v3.tile([P, R, 3], bf16, tag="rs1d", name="rs1d")
                s["RS2d"] = v3.tile([P, R, 3], bf16, tag="rs2d", name="rs2d")
                s["SD2"] = sc.tile([P, 2, R], f32, tag="sd2", name="sd2")
                s["IS1"] = sc.tile([P, R], f32, tag="is1", name="is1")
                s["G"] = sc.tile([P, R], f32, tag="g", name="g")
                s["S2"] = sc.tile([P, R], f32, tag="s2", name="s2")
                s["IS2"] = sc.tile([P, R], f32, tag="is2", name="is2")

            def alloc_tail(i):
                s = st[i]
                R = s["R"]
                s["E1"] = v3.tile([P, R, 3], bf16, tag="e1", name="e1")
                s["E2"] = v3.tile([P, R, 3], bf16, tag="e2", name="e2")
                s["SA"] = v3.tile([P, R, 3], bf16, tag="sa", name="sa")
                s["SB"] = v3.tile([P, R, 3], bf16, tag="sb", name="sb")
                s["TN"] = v3.tile([P, R, 3], bf16, tag="tn", name="tn")
                s["TO"] = v3.tile([P, R, 3], bf16, tag="to", name="to")
                s["OUT"] = io.tile([P, 3, R, 3], bf16, tag="OUT", name="OUT")

            def tail_a(i):
                # E1/E2 products + C-atom chain (fills ACT rs1 hop)
                if i < 0:
                    return
                s = st[i]
                nc.vector.tensor_mul(s["E1"], s["D1b"], s["RS1d"])
                nc.vector.tensor_mul(s["E2"], s["Wb"], s["RS2d"])
                nc.vector.tensor_scalar_mul(out=s["SA"], in0=s["E1"],
                                            scalar1=1.526)
                nc.vector.tensor_add(s["OUT"][:, 1], s["SA"], s["CAb"])

            def tail_b(i):
                # remaining placement chain (fills ACT sq2/wb hop)
                if i < 0:
                    return
                s = st[i]
                nc.vector.tensor_scalar_mul(out=s["SB"], in0=s["E2"],
                                            scalar1=1.363)
                nc.vector.tensor_add(s["TN"], s["SB"], s["CAb"])
                nc.vector.tensor_scalar_mul(out=s["SA"], in0=s["E1"],
                                            scalar1=-0.525)
                nc.vector.tensor_add(s["OUT"][:, 0], s["SA"], s["TN"])
                nc.vector.tensor_scalar_mul(out=s["SB"], in0=s["E2"],
                                            scalar1=-1.062)
                nc.vector.tensor_add(s["TO"], s["SB"], s["CAb"])
                nc.vector.tensor_scalar_mul(out=s["SA"], in0=s["E1"],
                                            scalar1=2.153)
                nc.vector.tensor_add(s["OUT"][:, 2], s["SA"], s["TO"])

            def emit_store(i, atom=None):
                if i < 0:
                    return
                s = st[i]
                if atom is None:
                    nc.scalar.dma_start(
                        out=Y[:, s["off"]:s["off"] + P * s["R"], :].rearrange(
                            "a (p r) c -> p a r c", p=P),
                        in_=s["OUT"])
                else:
                    nc.scalar.dma_start(
                        out=Y[atom, s["off"]:s["off"] + P * s["R"], :]
                        .rearrange("(p r) c -> p r c", p=P),
                        in_=s["OUT"][:, atom])

            # ---- prologue ----
            emit_load(0)
            emit_subs(0)

            for r in range(n):
                s = st[r]
                R = s["R"]
                alloc_head(r)
                if r > 0:
                    alloc_tail(r - 1)
                if r + 1 < n:
                    emit_load(r + 1)
                    emit_subs(r + 1)
                # s1 & dot (dot products in bf16 at 2x; squares f32 on ACT)
                nc.scalar.activation(out=s["QP"][:, 0], in_=s["D1"],
                                     func=SQUARE)
                nc.scalar.activation(out=s["CAb"], in_=s["T"][:, :, 3:6],
                                     func=COPY)
                nc.scalar.activation(out=s["D1b"], in_=s["D1"], func=COPY)
                nc.scalar.activation(out=s["Vb"], in_=s["V"], func=COPY)
                nc.vector.tensor_mul(s["QP"][:, 1], s["D1b"], s["Vb"])
                nc.vector.reduce_sum(out=s["SD2"], in_=s["QP"], axis=AX)
                nc.vector.reciprocal_approx_fast(out=s["IS1"],
                                                 in_=s["SD2"][:, 0])
                nc.scalar.activation(out=s["RS1d"], in_=bcast(s["IS1"], R),
                                     func=SQRT)
                # fill DVE while ACT does the rs1 chain
                tail_a(r - 1)
                # rejection
                nc.vector.tensor_mul(s["G"], s["SD2"][:, 1], s["IS1"])
                nc.vector.tensor_mul(s["T1"], s["D1"], bcast(s["G"], R))
                nc.vector.tensor_sub(s["W"], s["V"], s["T1"])
                nc.scalar.activation(out=s["SQ2"], in_=s["W"], func=SQUARE)
                nc.scalar.activation(out=s["Wb"], in_=s["W"], func=COPY)
                # fill DVE while ACT does sq2/wb
                tail_b(r - 1)
                emit_store(r - 1)
                nc.vector.reduce_sum(out=s["S2"], in_=s["SQ2"], axis=AX)
                nc.vector.reciprocal_approx_fast(out=s["IS2"], in_=s["S2"])
                nc.scalar.activation(out=s["RS2d"], in_=bcast(s["IS2"], R),
                                     func=SQRT)

            # ---- epilogue: last tile's tail (stores split per atom) ----
            alloc_tail(n - 1)
            tail_a(n - 1)
            emit_store(n - 1, atom=1)
            tail_b(n - 1)
            emit_store(n - 1, atom=0)
            emit_store(n - 1, atom=2)
    nc.finalize()
    return nc


def _get_nc():
    global _NC
    if _NC is None:
        _NC = _build_nc()
    return _NC


def make_in_maps(X):
    """Pack (N, CA, C) as contiguous [N_CORE, 9] f32 shards per core."""
    X = np.asarray(X, dtype=np.float32)
    A = np.ascontiguousarray(X[:, 0:3, :]).reshape(N_TOTAL, CIN)
    shards = A.reshape(N_CORES, N_CORE, CIN)
    return [{"X": shards[c]} for c in range(N_CORES)]


def assemble(X, results):
    """Stitch device outputs (atom-major bf16 [3, rows, 3]) into the
    full [N, 14, 3] f32 array."""
    out = np.array(X, dtype=np.float32, copy=True)
    Ys = [np.asarray(results[c]["Y"]).astype(np.float32)
          for c in range(N_CORES)]
    Yall = np.concatenate(Ys, axis=1)  # [3, N_TOTAL, 3]
    out[:, 0, :] = Yall[0]
    out[:, 2, :] = Yall[1]
    out[:, 3, :] = Yall[2]
    return out


def kernel(X, batch_ids=None, max_len=None, **_unused):
    from concourse.bass_utils import run_bass_kernel_spmd

    X = np.asarray(X, dtype=np.float32)
    assert X.shape == (N_TOTAL, 14, 3), X.shape
    nc = _get_nc()
    res = run_bass_kernel_spmd(nc, make_in_maps(X), list(range(N_CORES))).results
    return assemble(X, res)
